# revision 19
# baseline (speedup 1.0000x reference)
"""Trainium2 Bass kernel for AttentionStem (sparse local 4x4-window attention).

Contract: kernel(**inputs) takes the FULL unsharded inputs (numpy, keyed as in
setup_inputs()) and returns the FULL output [4, 64, 128, 128] float32.

Algorithm (per output pixel (b, h, w), per channel o):
    q  = query_w @ x                    (1x1 conv)
    kc = key_w @ xpad                   (1x1 conv on padded grid)
    vs_k = W_k @ xpad,  W_k = sum_m softmax_m(emb)[m,k] * value_w[m]
    attn_k = softmax_k(q * kc[pix + off_k])        (16 window positions)
    out = sum_k attn_k * vs_k[pix + off_k]

Sharding: 8 cores = 4 batches x 2 H-halves (64 output rows each, 4-row halo).
On-chip layout: [128 partitions = 2 row-blocks x 64 channels, free = pixels].

V2 engine assignment (vs the v1 baseline that was DVE+ACT bound):
  * All 1x1 convs use a 6-deep block-diagonal contraction (both row-blocks'
    3 input channels stacked on partitions 0-5) so ONE matmul produces both
    row-blocks -> half the PE streaming cycles of the 2-matmul version.
  * s = sum_k e_k and num = sum_k e_k*vs_k accumulate on the TensorEngine via
    identity matmuls into PSUM (fp32), start/stop accumulation groups across
    the 16 window positions. This removes the DVE ADD chain (~81us) and the
    bf16 accumulator error of v1.
  * The value-path products: vs_k stays in PSUM; for K_DIRECT groups/section
    the val-mul reads PSUM directly (DVE 1x), for the rest ScalarE evacuates
    vs to SBUF bf16 and the val-mul runs at DVE 2x. The split balances
    DVE vs ACT occupancy (tunable).
  * exp stays on ScalarE (only engine with LUTs), FD 4096 per call.
Softmax without max-subtraction: |logit| <= |q|*|k| stays well below exp
overflow for these Gaussian-scaled inputs.
"""

import os
import sys

import numpy as np

sys.path.insert(0, "/opt/trn_rl_repo")

# Problem constants (hardcoded; kernel.py must be self-contained).
B, IC, OC, H, W = 4, 3, 64, 128, 128
KS, PAD, M = 4, 2, 4
NCORES = 8

W132 = W + 2 * PAD  # padded width = 132
SH_OUT_ROWS = 64  # output rows per core
SH_ROWS = SH_OUT_ROWS + KS  # padded input rows per core = 68
BLK = 32  # output rows per partition-block
XP6_FREE = SH_ROWS * W132 + 16  # xp6 slab free size (+pad for shifted reads)
NR = 8  # output rows per section
NSEC = BLK // NR  # 4 sections
HFC = NR * W  # compact free elems per section = 1024
KCVR = NR + KS - 1  # kcv rows per section = 11
KCVF = KCVR * W132  # 1452
KCV_PAD = 16
NK = KS * KS

# pair = (k, k+4): same column offset j, row offsets i and i+1.
# group = 2 pairs -> 4 k-slots, one [128, 4*HFC] ep tile, one exp call.
# Group 0 uses only even shifts (j=0,2) so the first logits of a section
# never wait on the kcv1 shifted-twin DMA.
GROUPS = [
    [(0, 4), (2, 6)],
    [(1, 5), (3, 7)],
    [(8, 12), (10, 14)],
    [(9, 13), (11, 15)],
]

# Config knobs (tuned on hardware).
CFG = {
    # groups per section whose val-mul reads vs straight from PSUM (DVE 1x);
    # the other (4-K_DIRECT) groups get a ScalarE PSUM->SBUF bf16 evacuation
    # and a 2x DVE val-mul. Balances DVE vs ACT.
    "direct": int(os.environ.get("K_DIRECT", "2")),
}

_CACHE = {}
LAST_RESULT = None  # BassKernelResults of the most recent run (for test.py)


def _emit(nc, tc, aps, cfg):
    """Emit the per-core program.

    aps: xp6 [6, XP6_FREE] bf16, wts6 [6, 128*18] bf16, ident [128,128] bf16,
    out [64, 64, 128] f32.
    wts6 layout: slot 0 = q, slot 1 = k, slots 2+k = W_k; block-diagonal
    [0:3, 0:64] / [3:6, 64:128]."""
    from contextlib import ExitStack

    import concourse.bass as bass
    from concourse import mybir

    f32 = mybir.dt.float32
    bf16 = mybir.dt.bfloat16
    EXP = mybir.ActivationFunctionType.Exp
    NDIRECT = cfg["direct"]

    with ExitStack() as ctx:
        const = ctx.enter_context(tc.tile_pool(name="const", bufs=1))
        qkp = ctx.enter_context(tc.tile_pool(name="qk", bufs=2))
        epp = ctx.enter_context(tc.tile_pool(name="ep", bufs=2))
        scp = ctx.enter_context(tc.tile_pool(name="sc", bufs=2, space="PSUM"))
        accp = ctx.enter_context(tc.tile_pool(name="accps", bufs=1, space="PSUM"))
        finp = ctx.enter_context(tc.tile_pool(name="fin", bufs=2))

        # ---- load inputs ----
        xp6 = const.tile([6, XP6_FREE], bf16, tag="xp6", name="xp6")
        wts6 = const.tile([6, OC * 2 * 18], bf16, tag="wts6", name="wts6")
        ident = const.tile([128, 128], bf16, tag="ident", name="ident")
        # parallel queues so the first conv's inputs land ASAP
        nc.sync.dma_start(wts6[:], aps["wts6"][:])
        nc.sync.dma_start(ident[:], aps["ident"][:])
        # first piece covers section 0's conv reads (rows 0..13 of the slab)
        P1 = 14 * W132
        nc.scalar.dma_start(xp6[:, 0:P1], aps["xp6"][:, 0:P1])
        nc.gpsimd.dma_start(xp6[:, P1:XP6_FREE], aps["xp6"][:, P1:XP6_FREE])

        def wslot(idx):
            return wts6[:, 128 * idx : 128 * (idx + 1)]

        def conv(pt, dst_off, slot, i, j, row0, nrows):
            """pt[:, dst_off + r*W + w] = conv at out-rows [row0, row0+nrows),
            window offset (i, j); compact 128-wide output."""
            rr = 0
            while rr < nrows:
                sub = min(4, nrows - rr)
                base = (row0 + rr + i) * W132 + j
                rhs = xp6[:, base : base + sub * W132].rearrange(
                    "c (r w) -> c r w", w=W132
                )[:, :, 0:W]
                mm = nc.tensor.matmul(
                    pt[:, dst_off + rr * W : dst_off + (rr + sub) * W],
                    wslot(slot), rhs,
                )
                if rr > 0:
                    mm.ins.ldweights = False
                rr += sub

        def conv_flat(pt, dst_off, slot, base, n):
            """pt[:, dst_off:dst_off+n] = conv over flat xp6 range (132-layout
            output, used for kcv)."""
            off = 0
            while off < n:
                cn = min(512, n - off)
                mm = nc.tensor.matmul(
                    pt[:, dst_off + off : dst_off + off + cn],
                    wslot(slot),
                    xp6[:, base + off : base + off + cn],
                )
                if off > 0:
                    mm.ins.ldweights = False
                off += cn

        # ---- per-section q / kcv tiles ----
        def emit_qk(s, ramp=False):
            """q conv (compact) + kcv conv (132-layout) + kcv1 shifted twin."""
            evac = nc.vector.tensor_copy if ramp else nc.scalar.copy
            q_t = qkp.tile([128, HFC], bf16, tag="q", name="q")
            kcv0 = qkp.tile([128, KCVF + KCV_PAD], bf16, tag="kcv0", name="kcv0")
            kcv1 = qkp.tile([128, KCVF + KCV_PAD], bf16, tag="kcv1", name="kcv1")
            row0 = s * NR
            # q: window-center offset (2,2)
            ptq = scp.tile([128, HFC], f32, tag="sc", name="ptq")
            conv(ptq, 0, 0, PAD, PAD, row0, NR)
            evac(q_t[:], ptq[:])
            # kcv: flat 132-layout rows [row0, row0+11)
            pta = scp.tile([128, HFC], f32, tag="sc", name="pta")
            conv_flat(pta, 0, 1, row0 * W132, HFC)
            evac(kcv0[:, 0:HFC], pta[:])
            ptb = scp.tile([128, HFC], f32, tag="sc", name="ptb")
            conv_flat(ptb, 0, 1, row0 * W132 + HFC, KCVF - HFC)
            evac(kcv0[:, HFC:KCVF], ptb[:, 0 : KCVF - HFC])
            # kcv1[f] = kcv0[f+1] so odd window shifts keep 4B alignment
            # (reads never go past element 1449, so no pad memset needed)
            if ramp:
                nc.vector.tensor_copy(kcv1[:, 0 : KCVF - 1], kcv0[:, 1:KCVF])
            else:
                for p0 in range(0, 128, 32):
                    nc.sync.dma_start(
                        kcv1[p0 : p0 + 32, 0 : KCVF - 1],
                        kcv0[p0 : p0 + 32, 1:KCVF],
                    )
            return q_t, kcv0, kcv1

        def logit_pair(qt, kcv0, kcv1, ep, pi, ka):
            """ep slots [2pi, 2pi+1] = q * kcv[(i,j)], q * kcv[(i+1,j)] in one
            DVE op: kcv gets a [132,2] outer dim (rows i, i+1), q a
            broadcast dim."""
            i, j = ka // KS, ka % KS
            shift = i * W132 + j
            if shift % 2 == 1:
                ksrc, koff = kcv1, shift - 1
            else:
                ksrc, koff = kcv0, shift
            vk = ksrc[:, koff : koff + NR * W132].rearrange(
                "p (r w) -> p r w", w=W132
            )[:, :, 0:W]
            kk = bass.AP(vk.tensor, vk.offset,
                         [list(vk.ap)[0], [W132, 2], *list(vk.ap)[1:]])
            vq = qt[:].rearrange("p (r w) -> p r w", w=W)
            qq = bass.AP(vq.tensor, vq.offset,
                         [list(vq.ap)[0], [0, 2], *list(vq.ap)[1:]])
            out = ep[:, 2 * pi * HFC : (2 * pi + 2) * HFC].rearrange(
                "p (s r w) -> p s r w", s=2, w=W
            )
            nc.vector.tensor_mul(out, qq, kk)

        # ---- main loop ----

        def evac_accs(s_ps, num_ps):
            """ScalarE copies s/num out of PSUM right after the last acc
            burst so the banks free up for the next section without waiting
            on the DVE normalize chain."""
            s_sb = finp.tile([128, HFC], f32, tag="ssb", name="ssb")
            nc.scalar.copy(s_sb[:], s_ps[:])
            n_sb = finp.tile([128, HFC], f32, tag="nsb", name="nsb")
            nc.scalar.copy(n_sb[:], num_ps[:])
            return s_sb, n_sb

        def emit_finals(s, s_sb, n_sb):
            rinv = finp.tile([128, HFC], f32, tag="rinv", name="rinv")
            nc.vector.reciprocal_approx_fast(rinv[:], s_sb[:])
            o_t = finp.tile([128, HFC], f32, tag="o", name="o")
            nc.vector.tensor_mul(o_t[:], n_sb[:], rinv[:])
            o_v = o_t[:].rearrange("p (r w) -> p r w", w=W)
            for b in (0, 1):
                nc.sync.dma_start(
                    aps["out"][:, b * BLK + s * NR : b * BLK + (s + 1) * NR, :],
                    o_v[64 * b : 64 * (b + 1)],
                )

        def emit_accs(g, ep_g, p_g, s_ps, num_ps):
            """psum_s += I @ e, psum_num += I @ p for group g's tiles.
            The identity stays resident in the PE array across the burst."""
            first = True
            for src, dst in ((ep_g, s_ps), (p_g, num_ps)):
                for c in range(8):
                    mm = nc.tensor.matmul(
                        dst[:, (c % 2) * 512 : (c % 2 + 1) * 512],
                        ident[:],
                        src[:, c * 512 : (c + 1) * 512],
                        start=(g == 0 and c < 2),
                        stop=(g == 3 and c >= 6),
                        skip_group_check=True,
                    )
                    if not first:
                        mm.ins.ldweights = False
                    first = False

        # pending_accs: (g, ep_g, p_g, s_ps, num_ps) emitted one group late so
        # the acc burst fills the PE window where this group's later value
        # convs wait on PSUM-scratch reuse (keeps the PE p-state ramped).
        pending_accs = None
        pending_finals = None
        qk_cur = emit_qk(0, ramp=True)
        qk_next = None
        for s in range(NSEC):
            row0 = s * NR
            qt, kcv0, kcv1 = qk_cur

            ep = epp.tile([128, 4 * HFC], bf16, tag="ep", name="ep")
            for pi, (ka, _kb) in enumerate(GROUPS[0]):
                logit_pair(qt, kcv0, kcv1, ep, pi, ka)

            s_ps = num_ps = None
            for g in range(4):
                group = GROUPS[g]
                slots = [k for pair in group for k in pair]  # [ka,ka+4,kb,kb+4]
                # evac groups first: ScalarE has slack early in the section;
                # direct (PSUM-read) groups close the section so the last
                # val-muls never wait on ScalarE copies.
                direct = g >= 4 - NDIRECT
                nc.scalar.activation(ep[:], ep[:], EXP)

                # value convs -> PSUM, first half (scratch slots 0-1)
                vs_ps = []
                for idx in (0, 1):
                    k = slots[idx]
                    pt = scp.tile([128, HFC], f32, tag="sc", name="vs")
                    conv(pt, 0, 2 + k, k // KS, k % KS, row0, NR)
                    vs_ps.append(pt)

                # last group's acc burst: ready PE work that covers the
                # scratch-WAR stall before slots 2-3 can start
                if pending_accs is not None:
                    emit_accs(*pending_accs)
                    pending_accs = None
                if g == 0:
                    # prev section fully accumulated now: ScalarE-evacuate its
                    # s/num, re-claim the banks (alloc after the evac reads so
                    # WAR deps are tracked), then normalize at DVE leisure
                    if pending_finals is not None:
                        ps, old_sps, old_nps = pending_finals
                        s_sb, n_sb = evac_accs(old_sps, old_nps)
                        pending_finals = None
                    else:
                        ps = None
                    s_ps = accp.tile([128, HFC], f32, tag="s", name="s_ps")
                    num_ps = accp.tile([128, HFC], f32, tag="num", name="num_ps")
                    if ps is not None:
                        emit_finals(ps, s_sb, n_sb)

                # value convs, second half
                for idx in (2, 3):
                    k = slots[idx]
                    pt = scp.tile([128, HFC], f32, tag="sc", name="vs")
                    conv(pt, 0, 2 + k, k // KS, k % KS, row0, NR)
                    vs_ps.append(pt)

                # next group's logits (DVE) ahead of this group's val-mul
                ep_prev = ep
                if g < 3:
                    ep = epp.tile([128, 4 * HFC], bf16, tag="ep", name="ep")
                    for pi, (ka, _kb) in enumerate(GROUPS[g + 1]):
                        logit_pair(qt, kcv0, kcv1, ep, pi, ka)
                if g == 1 and s + 1 < NSEC:
                    # next section's q/kcv early: plenty of ScalarE slack
                    # mid-section, and the kcv1 twin DMA gets lead time
                    qk_next = emit_qk(s + 1)

                # val-mul: p = e * vs, per k-slot so each mul waits on only
                # one evacuation / conv
                p_t = epp.tile([128, 4 * HFC], bf16, tag="p", name="p")
                vse = None
                if not direct:
                    vse = epp.tile([128, 4 * HFC], bf16, tag="vse", name="vse")
                for idx in range(4):
                    if direct:
                        src = vs_ps[idx][:]
                    else:
                        nc.scalar.copy(
                            vse[:, idx * HFC : (idx + 1) * HFC], vs_ps[idx][:]
                        )
                        src = vse[:, idx * HFC : (idx + 1) * HFC]
                    nc.vector.tensor_mul(
                        p_t[:, idx * HFC : (idx + 1) * HFC],
                        ep_prev[:, idx * HFC : (idx + 1) * HFC],
                        src,
                    )

                pending_accs = (g, ep_prev, p_t, s_ps, num_ps)

            pending_finals = (s, s_ps, num_ps)
            if qk_next is not None:
                qk_cur, qk_next = qk_next, None

        emit_accs(*pending_accs)
        ps, old_sps, old_nps = pending_finals
        s_sb, n_sb = evac_accs(old_sps, old_nps)
        emit_finals(ps, s_sb, n_sb)


def _build(cfg):
    key = tuple(sorted(cfg.items()))
    if key in _CACHE:
        return _CACHE[key]
    import concourse.tile as tile
    from concourse import bacc, mybir

    nc = bacc.Bacc(
        "TRN2", target_bir_lowering=False, debug=False, num_devices=NCORES
    )
    f32 = mybir.dt.float32
    bf16 = mybir.dt.bfloat16
    aps = {}
    aps["xp6"] = nc.dram_tensor("xp6", [6, XP6_FREE], bf16,
                                kind="ExternalInput").ap()
    aps["wts6"] = nc.dram_tensor("wts6", [6, OC * 2 * 18], bf16,
                                 kind="ExternalInput").ap()
    aps["ident"] = nc.dram_tensor("ident", [128, 128], bf16,
                                  kind="ExternalInput").ap()
    aps["out"] = nc.dram_tensor("out", [OC, SH_OUT_ROWS, W], f32,
                                kind="ExternalOutput").ap()

    with tile.TileContext(nc) as tc:
        _emit(nc, tc, aps, cfg)
    nc.compile()
    _CACHE[key] = nc
    return nc


def _host_prep(inputs, cfg):
    import ml_dtypes

    x = np.asarray(inputs["x"], np.float32)
    key_w = np.asarray(inputs["key_w"], np.float32)
    query_w = np.asarray(inputs["query_w"], np.float32)
    value_w = np.asarray(inputs["value_w"], np.float32)
    emb_a = np.asarray(inputs["emb_a"], np.float32)
    emb_b = np.asarray(inputs["emb_b"], np.float32)
    emb_mix = np.asarray(inputs["emb_mix"], np.float32)

    # emb softmax over m, then effective per-offset value matrices W_k [16,64,3]
    la = emb_mix @ emb_a  # (M, KS)
    lb = emb_mix @ emb_b  # (M, KS)
    eloG = (la[:, :, None] + lb[:, None, :]).reshape(M, KS * KS).astype(np.float64)
    eloG -= eloG.max(axis=0, keepdims=True)
    emb = np.exp(eloG)
    emb /= emb.sum(axis=0, keepdims=True)  # (M, 16)
    wk = np.einsum("mk,moc->koc", emb.astype(np.float32), value_w)  # (16,64,3)

    # wts6 [6, 128*18]: block-diagonal (block0 chans on rows 0-2 -> parts 0-63,
    # block1 chans on rows 3-5 -> parts 64-127)
    wts6 = np.zeros((6, 128 * 18), np.float32)
    mats = [query_w, key_w] + [wk[k] for k in range(NK)]
    for idx, mat in enumerate(mats):
        wts6[0:3, 128 * idx : 128 * idx + 64] = mat.T
        wts6[3:6, 128 * idx + 64 : 128 * (idx + 1)] = mat.T

    ident = np.eye(128, dtype=np.float32)

    # padded input, shards
    xp = np.zeros((B, IC, H + 2 * PAD, W + 2 * PAD), np.float32)
    xp[:, :, PAD : PAD + H, PAD : PAD + W] = x

    in_maps = []
    for c in range(NCORES):
        b, hh = c // 2, c % 2
        sh = xp[b, :, hh * SH_OUT_ROWS : hh * SH_OUT_ROWS + SH_ROWS, :]
        slab6 = np.zeros((6, XP6_FREE), np.float32)
        slab6[0:3, : SH_ROWS * W132] = sh.reshape(IC, -1)
        # rows 32.. (block1's conv window) shifted onto partitions 3-5
        slab6[3:6, : (SH_ROWS - BLK) * W132] = sh[:, BLK:, :].reshape(IC, -1)
        in_maps.append({
            "xp6": slab6.astype(ml_dtypes.bfloat16),
            "wts6": wts6.astype(ml_dtypes.bfloat16),
            "ident": ident.astype(ml_dtypes.bfloat16),
        })
    return in_maps


def _ensure_ntff_hook():
    """The agent image's antenv lacks axon_hooks, so boot() could not register
    the NTFF profile hook. Inject the registry module and register the
    ctypes-based hook so run_bass_kernel_spmd(trace=True) can profile."""
    import types

    try:
        import antenv
    except ImportError:
        return
    if "antenv.axon_hooks" in sys.modules:
        return
    try:
        from trn_agent_boot.trn_boot import _ntff_profile_via_ctypes

        hook = _ntff_profile_via_ctypes("/opt/axon/libaxon_pjrt.so")
    except Exception:
        hook = None
    mod = types.ModuleType("antenv.axon_hooks")
    mod._hook = hook
    mod.set_axon_ntff_profile_hook = lambda h: setattr(mod, "_hook", h)
    mod.get_axon_ntff_profile_hook = lambda: mod._hook
    sys.modules["antenv.axon_hooks"] = mod
    antenv.axon_hooks = mod


def kernel(**inputs):
    global LAST_RESULT
    cfg = dict(CFG)
    in_maps = _host_prep(inputs, cfg)
    nc = _build(cfg)

    from concourse.bass_utils import run_bass_kernel_spmd

    trace = os.environ.get("KERNEL_TRACE", "0") == "1"
    if trace:
        _ensure_ntff_hook()
    res = run_bass_kernel_spmd(
        nc, in_maps, core_ids=list(range(NCORES)), trace=trace
    )
    LAST_RESULT = res

    out = np.empty((B, OC, H, W), np.float32)
    for c in range(NCORES):
        b, hh = c // 2, c % 2
        out[b, :, hh * SH_OUT_ROWS : (hh + 1) * SH_OUT_ROWS, :] = res.results[c]["out"]
    return out


# revision 22
# speedup vs baseline: 1.0557x; 1.0557x over previous
"""Trainium2 Bass kernel for AttentionStem (sparse local 4x4-window attention).

Contract: kernel(**inputs) takes the FULL unsharded inputs (numpy, keyed as in
setup_inputs()) and returns the FULL output [4, 64, 128, 128] float32.

Algorithm (per output pixel (b, h, w), per channel o):
    q  = query_w @ x                    (1x1 conv)
    kc = key_w @ xpad                   (1x1 conv on padded grid)
    vs_k = W_k @ xpad,  W_k = sum_m softmax_m(emb)[m,k] * value_w[m]
    attn_k = softmax_k(q * kc[pix + off_k])        (16 window positions)
    out = sum_k attn_k * vs_k[pix + off_k]

Sharding: 8 cores = 4 batches x 2 H-halves (64 output rows each, 4-row halo).
On-chip layout: [128 partitions = 2 row-blocks x 64 channels, free = pixels].

V2 engine assignment (vs the v1 baseline that was DVE+ACT bound):
  * All 1x1 convs use a 6-deep block-diagonal contraction (both row-blocks'
    3 input channels stacked on partitions 0-5) so ONE matmul produces both
    row-blocks -> half the PE streaming cycles of the 2-matmul version.
  * s = sum_k e_k and num = sum_k e_k*vs_k accumulate on the TensorEngine via
    identity matmuls into PSUM (fp32), start/stop accumulation groups across
    the 16 window positions. This removes the DVE ADD chain (~81us) and the
    bf16 accumulator error of v1.
  * The value-path products: vs_k stays in PSUM; for K_DIRECT groups/section
    the val-mul reads PSUM directly (DVE 1x), for the rest ScalarE evacuates
    vs to SBUF bf16 and the val-mul runs at DVE 2x. The split balances
    DVE vs ACT occupancy (tunable).
  * exp stays on ScalarE (only engine with LUTs), FD 4096 per call.
Softmax without max-subtraction: |logit| <= |q|*|k| stays well below exp
overflow for these Gaussian-scaled inputs.
"""

import os
import sys

import numpy as np

sys.path.insert(0, "/opt/trn_rl_repo")

# Problem constants (hardcoded; kernel.py must be self-contained).
B, IC, OC, H, W = 4, 3, 64, 128, 128
KS, PAD, M = 4, 2, 4
NCORES = 8

W132 = W + 2 * PAD  # padded width = 132
SH_OUT_ROWS = 64  # output rows per core
SH_ROWS = SH_OUT_ROWS + KS  # padded input rows per core = 68
BLK = 32  # output rows per partition-block
XP6_FREE = SH_ROWS * W132 + 16  # xp6 slab free size (+pad for shifted reads)
NR = 8  # output rows per section
NSEC = BLK // NR  # 4 sections
HFC = NR * W  # compact free elems per section = 1024
KCVR = NR + KS - 1  # kcv rows per section = 11
KCVF = KCVR * W132  # 1452
KCV_PAD = 16
NK = KS * KS

# pair = (k, k+4): same column offset j, row offsets i and i+1.
# group = 2 pairs -> 4 k-slots, one [128, 4*HFC] ep tile, one exp call.
# Group 0 uses only even shifts (j=0,2) so the first logits of a section
# never wait on the kcv1 shifted-twin DMA.
GROUPS = [
    [(0, 4), (2, 6)],
    [(1, 5), (3, 7)],
    [(8, 12), (10, 14)],
    [(9, 13), (11, 15)],
]

# Config knobs (tuned on hardware).
CFG = {
    # groups per section whose val-mul reads vs straight from PSUM (DVE 1x);
    # the other (4-K_DIRECT) groups get a ScalarE PSUM->SBUF bf16 evacuation
    # and a 2x DVE val-mul. Balances DVE vs ACT.
    "direct": int(os.environ.get("K_DIRECT", "2")),
    # which group iteration emits the next section's q/kcv convs+evacs
    "qk_at": int(os.environ.get("K_QK_AT", "3")),
}

_CACHE = {}
LAST_RESULT = None  # BassKernelResults of the most recent run (for test.py)


def _emit(nc, tc, aps, cfg):
    """Emit the per-core program.

    aps: xp6 [6, XP6_FREE] bf16, wts6 [6, 128*18] bf16, ident [128,128] bf16,
    out [64, 64, 128] f32.
    wts6 layout: slot 0 = q, slot 1 = k, slots 2+k = W_k; block-diagonal
    [0:3, 0:64] / [3:6, 64:128]."""
    from contextlib import ExitStack

    import concourse.bass as bass
    from concourse import mybir

    f32 = mybir.dt.float32
    bf16 = mybir.dt.bfloat16
    EXP = mybir.ActivationFunctionType.Exp
    NDIRECT = cfg["direct"]
    QK_AT = cfg["qk_at"]

    with ExitStack() as ctx:
        const = ctx.enter_context(tc.tile_pool(name="const", bufs=1))
        qkp = ctx.enter_context(tc.tile_pool(name="qk", bufs=2))
        epp = ctx.enter_context(tc.tile_pool(name="ep", bufs=2))
        scp = ctx.enter_context(tc.tile_pool(name="sc", bufs=2, space="PSUM"))
        accp = ctx.enter_context(tc.tile_pool(name="accps", bufs=1, space="PSUM"))
        finp = ctx.enter_context(tc.tile_pool(name="fin", bufs=2))

        # ---- load inputs ----
        xp6 = const.tile([6, XP6_FREE], bf16, tag="xp6", name="xp6")
        wts6 = const.tile([6, OC * 2 * 18], bf16, tag="wts6", name="wts6")
        ident = const.tile([128, 128], bf16, tag="ident", name="ident")
        # parallel queues so the first conv's inputs land ASAP
        nc.sync.dma_start(wts6[:], aps["wts6"][:])
        nc.sync.dma_start(ident[:], aps["ident"][:])
        # first piece covers section 0's conv reads (rows 0..13 of the slab)
        P1 = 14 * W132
        nc.scalar.dma_start(xp6[:, 0:P1], aps["xp6"][:, 0:P1])
        nc.gpsimd.dma_start(xp6[:, P1:XP6_FREE], aps["xp6"][:, P1:XP6_FREE])

        def wslot(idx):
            return wts6[:, 128 * idx : 128 * (idx + 1)]

        def conv(pt, dst_off, slot, i, j, row0, nrows):
            """pt[:, dst_off + r*W + w] = conv at out-rows [row0, row0+nrows),
            window offset (i, j); compact 128-wide output."""
            rr = 0
            while rr < nrows:
                sub = min(4, nrows - rr)
                base = (row0 + rr + i) * W132 + j
                rhs = xp6[:, base : base + sub * W132].rearrange(
                    "c (r w) -> c r w", w=W132
                )[:, :, 0:W]
                mm = nc.tensor.matmul(
                    pt[:, dst_off + rr * W : dst_off + (rr + sub) * W],
                    wslot(slot), rhs,
                )
                if rr > 0:
                    mm.ins.ldweights = False
                rr += sub

        def conv_flat(pt, dst_off, slot, base, n):
            """pt[:, dst_off:dst_off+n] = conv over flat xp6 range (132-layout
            output, used for kcv)."""
            off = 0
            while off < n:
                cn = min(512, n - off)
                mm = nc.tensor.matmul(
                    pt[:, dst_off + off : dst_off + off + cn],
                    wslot(slot),
                    xp6[:, base + off : base + off + cn],
                )
                if off > 0:
                    mm.ins.ldweights = False
                off += cn

        # ---- per-section q / kcv tiles ----
        def emit_qk(s, ramp=False):
            """q conv (compact) + kcv conv (132-layout) + kcv1 shifted twin."""
            evac = nc.vector.tensor_copy if ramp else nc.scalar.copy
            q_t = qkp.tile([128, HFC], bf16, tag="q", name="q")
            kcv0 = qkp.tile([128, KCVF + KCV_PAD], bf16, tag="kcv0", name="kcv0")
            kcv1 = qkp.tile([128, KCVF + KCV_PAD], bf16, tag="kcv1", name="kcv1")
            row0 = s * NR
            # q: window-center offset (2,2)
            ptq = scp.tile([128, HFC], f32, tag="sc", name="ptq")
            conv(ptq, 0, 0, PAD, PAD, row0, NR)
            evac(q_t[:], ptq[:])
            # kcv: flat 132-layout rows [row0, row0+11)
            pta = scp.tile([128, HFC], f32, tag="sc", name="pta")
            conv_flat(pta, 0, 1, row0 * W132, HFC)
            evac(kcv0[:, 0:HFC], pta[:])
            ptb = scp.tile([128, HFC], f32, tag="sc", name="ptb")
            conv_flat(ptb, 0, 1, row0 * W132 + HFC, KCVF - HFC)
            evac(kcv0[:, HFC:KCVF], ptb[:, 0 : KCVF - HFC])
            # kcv1[f] = kcv0[f+1] so odd window shifts keep 4B alignment
            # (reads never go past element 1449, so no pad memset needed)
            if ramp:
                nc.vector.tensor_copy(kcv1[:, 0 : KCVF - 1], kcv0[:, 1:KCVF])
            else:
                for p0 in range(0, 128, 32):
                    nc.sync.dma_start(
                        kcv1[p0 : p0 + 32, 0 : KCVF - 1],
                        kcv0[p0 : p0 + 32, 1:KCVF],
                    )
            return q_t, kcv0, kcv1

        def logit_pair(qt, kcv0, kcv1, ep, pi, ka):
            """ep slots [2pi, 2pi+1] = q * kcv[(i,j)], q * kcv[(i+1,j)] in one
            DVE op: kcv gets a [132,2] outer dim (rows i, i+1), q a
            broadcast dim."""
            i, j = ka // KS, ka % KS
            shift = i * W132 + j
            if shift % 2 == 1:
                ksrc, koff = kcv1, shift - 1
            else:
                ksrc, koff = kcv0, shift
            vk = ksrc[:, koff : koff + NR * W132].rearrange(
                "p (r w) -> p r w", w=W132
            )[:, :, 0:W]
            kk = bass.AP(vk.tensor, vk.offset,
                         [list(vk.ap)[0], [W132, 2], *list(vk.ap)[1:]])
            vq = qt[:].rearrange("p (r w) -> p r w", w=W)
            qq = bass.AP(vq.tensor, vq.offset,
                         [list(vq.ap)[0], [0, 2], *list(vq.ap)[1:]])
            out = ep[:, 2 * pi * HFC : (2 * pi + 2) * HFC].rearrange(
                "p (s r w) -> p s r w", s=2, w=W
            )
            nc.vector.tensor_mul(out, qq, kk)

        # ---- main loop ----

        def evac_accs(s_ps, num_ps):
            """ScalarE copies s/num out of PSUM right after the last acc
            burst so the banks free up for the next section without waiting
            on the DVE normalize chain."""
            s_sb = finp.tile([128, HFC], f32, tag="ssb", name="ssb")
            nc.scalar.copy(s_sb[:], s_ps[:])
            n_sb = finp.tile([128, HFC], f32, tag="nsb", name="nsb")
            nc.scalar.copy(n_sb[:], num_ps[:])
            return s_sb, n_sb

        def emit_finals(s, s_sb, n_sb):
            rinv = finp.tile([128, HFC], f32, tag="rinv", name="rinv")
            nc.vector.reciprocal_approx_fast(rinv[:], s_sb[:])
            o_t = finp.tile([128, HFC], f32, tag="o", name="o")
            nc.vector.tensor_mul(o_t[:], n_sb[:], rinv[:])
            o_v = o_t[:].rearrange("p (r w) -> p r w", w=W)
            for b in (0, 1):
                nc.sync.dma_start(
                    aps["out"][:, b * BLK + s * NR : b * BLK + (s + 1) * NR, :],
                    o_v[64 * b : 64 * (b + 1)],
                )

        def emit_accs(g, ep_g, p_g, s_ps, num_ps):
            """psum_s += I @ e, psum_num += I @ p for group g's tiles.
            The identity stays resident in the PE array across the burst."""
            first = True
            for src, dst in ((ep_g, s_ps), (p_g, num_ps)):
                for c in range(8):
                    mm = nc.tensor.matmul(
                        dst[:, (c % 2) * 512 : (c % 2 + 1) * 512],
                        ident[:],
                        src[:, c * 512 : (c + 1) * 512],
                        start=(g == 0 and c < 2),
                        stop=(g == 3 and c >= 6),
                        skip_group_check=True,
                    )
                    if not first:
                        mm.ins.ldweights = False
                    first = False

        # pending_accs: (g, ep_g, p_g, s_ps, num_ps) emitted one group late so
        # the acc burst fills the PE window where this group's later value
        # convs wait on PSUM-scratch reuse (keeps the PE p-state ramped).
        pending_accs = None
        pending_finals = None
        qk_cur = emit_qk(0, ramp=True)
        qk_next = None
        for s in range(NSEC):
            row0 = s * NR
            qt, kcv0, kcv1 = qk_cur

            ep = epp.tile([128, 4 * HFC], bf16, tag="ep", name="ep")
            for pi, (ka, _kb) in enumerate(GROUPS[0]):
                logit_pair(qt, kcv0, kcv1, ep, pi, ka)

            s_ps = num_ps = None
            for g in range(4):
                group = GROUPS[g]
                slots = [k for pair in group for k in pair]  # [ka,ka+4,kb,kb+4]
                # evac groups first: ScalarE has slack early in the section;
                # direct (PSUM-read) groups close the section so the last
                # val-muls never wait on ScalarE copies.
                direct = g >= 4 - NDIRECT
                nc.scalar.activation(ep[:], ep[:], EXP)

                # value convs -> PSUM, first half (scratch slots 0-1)
                vs_ps = []
                for idx in (0, 1):
                    k = slots[idx]
                    pt = scp.tile([128, HFC], f32, tag="sc", name="vs")
                    conv(pt, 0, 2 + k, k // KS, k % KS, row0, NR)
                    vs_ps.append(pt)

                # last group's acc burst: ready PE work that covers the
                # scratch-WAR stall before slots 2-3 can start
                if pending_accs is not None:
                    emit_accs(*pending_accs)
                    pending_accs = None
                if g == 0:
                    # prev section fully accumulated now: ScalarE-evacuate its
                    # s/num, re-claim the banks (alloc after the evac reads so
                    # WAR deps are tracked), then normalize at DVE leisure
                    if pending_finals is not None:
                        ps, old_sps, old_nps = pending_finals
                        s_sb, n_sb = evac_accs(old_sps, old_nps)
                        pending_finals = None
                    else:
                        ps = None
                    s_ps = accp.tile([128, HFC], f32, tag="s", name="s_ps")
                    num_ps = accp.tile([128, HFC], f32, tag="num", name="num_ps")
                    if ps is not None:
                        emit_finals(ps, s_sb, n_sb)

                # value convs, second half
                for idx in (2, 3):
                    k = slots[idx]
                    pt = scp.tile([128, HFC], f32, tag="sc", name="vs")
                    conv(pt, 0, 2 + k, k // KS, k % KS, row0, NR)
                    vs_ps.append(pt)

                # next group's logits (DVE) ahead of this group's val-mul
                ep_prev = ep
                if g < 3:
                    ep = epp.tile([128, 4 * HFC], bf16, tag="ep", name="ep")
                    for pi, (ka, _kb) in enumerate(GROUPS[g + 1]):
                        logit_pair(qt, kcv0, kcv1, ep, pi, ka)
                if g == QK_AT and s + 1 < NSEC:
                    qk_next = emit_qk(s + 1)

                # val-mul: p = e * vs, per k-slot so each mul waits on only
                # one evacuation / conv
                p_t = epp.tile([128, 4 * HFC], bf16, tag="p", name="p")
                vse = None
                if not direct:
                    vse = epp.tile([128, 4 * HFC], bf16, tag="vse", name="vse")
                for idx in range(4):
                    if direct:
                        src = vs_ps[idx][:]
                    else:
                        nc.scalar.copy(
                            vse[:, idx * HFC : (idx + 1) * HFC], vs_ps[idx][:]
                        )
                        src = vse[:, idx * HFC : (idx + 1) * HFC]
                    nc.vector.tensor_mul(
                        p_t[:, idx * HFC : (idx + 1) * HFC],
                        ep_prev[:, idx * HFC : (idx + 1) * HFC],
                        src,
                    )

                pending_accs = (g, ep_prev, p_t, s_ps, num_ps)

            pending_finals = (s, s_ps, num_ps)
            if qk_next is not None:
                qk_cur, qk_next = qk_next, None

        emit_accs(*pending_accs)
        ps, old_sps, old_nps = pending_finals
        s_sb, n_sb = evac_accs(old_sps, old_nps)
        emit_finals(ps, s_sb, n_sb)


def _build(cfg):
    key = tuple(sorted(cfg.items()))
    if key in _CACHE:
        return _CACHE[key]
    import concourse.tile as tile
    from concourse import bacc, mybir

    nc = bacc.Bacc(
        "TRN2", target_bir_lowering=False, debug=False, num_devices=NCORES
    )
    f32 = mybir.dt.float32
    bf16 = mybir.dt.bfloat16
    aps = {}
    aps["xp6"] = nc.dram_tensor("xp6", [6, XP6_FREE], bf16,
                                kind="ExternalInput").ap()
    aps["wts6"] = nc.dram_tensor("wts6", [6, OC * 2 * 18], bf16,
                                 kind="ExternalInput").ap()
    aps["ident"] = nc.dram_tensor("ident", [128, 128], bf16,
                                  kind="ExternalInput").ap()
    aps["out"] = nc.dram_tensor("out", [OC, SH_OUT_ROWS, W], f32,
                                kind="ExternalOutput").ap()

    with tile.TileContext(nc) as tc:
        _emit(nc, tc, aps, cfg)
    nc.compile()
    _CACHE[key] = nc
    return nc


def _host_prep(inputs, cfg):
    import ml_dtypes

    x = np.asarray(inputs["x"], np.float32)
    key_w = np.asarray(inputs["key_w"], np.float32)
    query_w = np.asarray(inputs["query_w"], np.float32)
    value_w = np.asarray(inputs["value_w"], np.float32)
    emb_a = np.asarray(inputs["emb_a"], np.float32)
    emb_b = np.asarray(inputs["emb_b"], np.float32)
    emb_mix = np.asarray(inputs["emb_mix"], np.float32)

    # emb softmax over m, then effective per-offset value matrices W_k [16,64,3]
    la = emb_mix @ emb_a  # (M, KS)
    lb = emb_mix @ emb_b  # (M, KS)
    eloG = (la[:, :, None] + lb[:, None, :]).reshape(M, KS * KS).astype(np.float64)
    eloG -= eloG.max(axis=0, keepdims=True)
    emb = np.exp(eloG)
    emb /= emb.sum(axis=0, keepdims=True)  # (M, 16)
    wk = np.einsum("mk,moc->koc", emb.astype(np.float32), value_w)  # (16,64,3)

    # wts6 [6, 128*18]: block-diagonal (block0 chans on rows 0-2 -> parts 0-63,
    # block1 chans on rows 3-5 -> parts 64-127)
    wts6 = np.zeros((6, 128 * 18), np.float32)
    mats = [query_w, key_w] + [wk[k] for k in range(NK)]
    for idx, mat in enumerate(mats):
        wts6[0:3, 128 * idx : 128 * idx + 64] = mat.T
        wts6[3:6, 128 * idx + 64 : 128 * (idx + 1)] = mat.T

    ident = np.eye(128, dtype=np.float32)

    # padded input, shards
    xp = np.zeros((B, IC, H + 2 * PAD, W + 2 * PAD), np.float32)
    xp[:, :, PAD : PAD + H, PAD : PAD + W] = x

    in_maps = []
    for c in range(NCORES):
        b, hh = c // 2, c % 2
        sh = xp[b, :, hh * SH_OUT_ROWS : hh * SH_OUT_ROWS + SH_ROWS, :]
        slab6 = np.zeros((6, XP6_FREE), np.float32)
        slab6[0:3, : SH_ROWS * W132] = sh.reshape(IC, -1)
        # rows 32.. (block1's conv window) shifted onto partitions 3-5
        slab6[3:6, : (SH_ROWS - BLK) * W132] = sh[:, BLK:, :].reshape(IC, -1)
        in_maps.append({
            "xp6": slab6.astype(ml_dtypes.bfloat16),
            "wts6": wts6.astype(ml_dtypes.bfloat16),
            "ident": ident.astype(ml_dtypes.bfloat16),
        })
    return in_maps


def _ensure_ntff_hook():
    """The agent image's antenv lacks axon_hooks, so boot() could not register
    the NTFF profile hook. Inject the registry module and register the
    ctypes-based hook so run_bass_kernel_spmd(trace=True) can profile."""
    import types

    try:
        import antenv
    except ImportError:
        return
    if "antenv.axon_hooks" in sys.modules:
        return
    try:
        from trn_agent_boot.trn_boot import _ntff_profile_via_ctypes

        hook = _ntff_profile_via_ctypes("/opt/axon/libaxon_pjrt.so")
    except Exception:
        hook = None
    mod = types.ModuleType("antenv.axon_hooks")
    mod._hook = hook
    mod.set_axon_ntff_profile_hook = lambda h: setattr(mod, "_hook", h)
    mod.get_axon_ntff_profile_hook = lambda: mod._hook
    sys.modules["antenv.axon_hooks"] = mod
    antenv.axon_hooks = mod


def kernel(**inputs):
    global LAST_RESULT
    cfg = dict(CFG)
    in_maps = _host_prep(inputs, cfg)
    nc = _build(cfg)

    from concourse.bass_utils import run_bass_kernel_spmd

    trace = os.environ.get("KERNEL_TRACE", "0") == "1"
    if trace:
        _ensure_ntff_hook()
    res = run_bass_kernel_spmd(
        nc, in_maps, core_ids=list(range(NCORES)), trace=trace
    )
    LAST_RESULT = res

    out = np.empty((B, OC, H, W), np.float32)
    for c in range(NCORES):
        b, hh = c // 2, c % 2
        out[b, :, hh * SH_OUT_ROWS : (hh + 1) * SH_OUT_ROWS, :] = res.results[c]["out"]
    return out


# revision 24
# speedup vs baseline: 1.0725x; 1.0159x over previous
"""Trainium2 Bass kernel for AttentionStem (sparse local 4x4-window attention).

Contract: kernel(**inputs) takes the FULL unsharded inputs (numpy, keyed as in
setup_inputs()) and returns the FULL output [4, 64, 128, 128] float32.

Algorithm (per output pixel (b, h, w), per channel o):
    q  = query_w @ x                    (1x1 conv)
    kc = key_w @ xpad                   (1x1 conv on padded grid)
    vs_k = W_k @ xpad,  W_k = sum_m softmax_m(emb)[m,k] * value_w[m]
    attn_k = softmax_k(q * kc[pix + off_k])        (16 window positions)
    out = sum_k attn_k * vs_k[pix + off_k]

Sharding: 8 cores = 4 batches x 2 H-halves (64 output rows each, 4-row halo).
On-chip layout: [128 partitions = 2 row-blocks x 64 channels, free = pixels].

V2 engine assignment (vs the v1 baseline that was DVE+ACT bound):
  * All 1x1 convs use a 6-deep block-diagonal contraction (both row-blocks'
    3 input channels stacked on partitions 0-5) so ONE matmul produces both
    row-blocks -> half the PE streaming cycles of the 2-matmul version.
  * s = sum_k e_k and num = sum_k e_k*vs_k accumulate on the TensorEngine via
    identity matmuls into PSUM (fp32), start/stop accumulation groups across
    the 16 window positions. This removes the DVE ADD chain (~81us) and the
    bf16 accumulator error of v1.
  * The value-path products: vs_k stays in PSUM; for K_DIRECT groups/section
    the val-mul reads PSUM directly (DVE 1x), for the rest ScalarE evacuates
    vs to SBUF bf16 and the val-mul runs at DVE 2x. The split balances
    DVE vs ACT occupancy (tunable).
  * exp stays on ScalarE (only engine with LUTs), FD 4096 per call.
Softmax without max-subtraction: |logit| <= |q|*|k| stays well below exp
overflow for these Gaussian-scaled inputs.
"""

import os
import sys

import numpy as np

sys.path.insert(0, "/opt/trn_rl_repo")

# Problem constants (hardcoded; kernel.py must be self-contained).
B, IC, OC, H, W = 4, 3, 64, 128, 128
KS, PAD, M = 4, 2, 4
NCORES = 8

W132 = W + 2 * PAD  # padded width = 132
SH_OUT_ROWS = 64  # output rows per core
SH_ROWS = SH_OUT_ROWS + KS  # padded input rows per core = 68
BLK = 32  # output rows per partition-block
XP6_FREE = SH_ROWS * W132 + 16  # xp6 slab free size (+pad for shifted reads)
NR = 8  # output rows per section
NSEC = BLK // NR  # 4 sections
HFC = NR * W  # compact free elems per section = 1024
KCVR = NR + KS - 1  # kcv rows per section = 11
KCVF = KCVR * W132  # 1452
KCV_PAD = 16
NK = KS * KS

# pair = (k, k+4): same column offset j, row offsets i and i+1.
# group = 2 pairs -> 4 k-slots, one [128, 4*HFC] ep tile, one exp call.
# Group 0 uses only even shifts (j=0,2) so the first logits of a section
# never wait on the kcv1 shifted-twin DMA.
GROUPS = [
    [(0, 4), (2, 6)],
    [(1, 5), (3, 7)],
    [(8, 12), (10, 14)],
    [(9, 13), (11, 15)],
]

# Config knobs (tuned on hardware).
CFG = {
    # groups per section whose val-mul reads vs straight from PSUM (DVE 1x);
    # the other (4-K_DIRECT) groups get a ScalarE PSUM->SBUF bf16 evacuation
    # and a 2x DVE val-mul. Balances DVE vs ACT.
    "direct": int(os.environ.get("K_DIRECT", "2")),
    # which group iteration emits the next section's q/kcv convs+evacs
    "qk_at": int(os.environ.get("K_QK_AT", "3")),
}

_CACHE = {}
LAST_RESULT = None  # BassKernelResults of the most recent run (for test.py)


def _emit(nc, tc, aps, cfg):
    """Emit the per-core program.

    aps: xp6 [6, XP6_FREE] bf16, wts6 [6, 128*18] bf16, ident [128,128] bf16,
    out [64, 64, 128] f32.
    wts6 layout: slot 0 = q, slot 1 = k, slots 2+k = W_k; block-diagonal
    [0:3, 0:64] / [3:6, 64:128]."""
    from contextlib import ExitStack

    import concourse.bass as bass
    from concourse import mybir

    f32 = mybir.dt.float32
    bf16 = mybir.dt.bfloat16
    EXP = mybir.ActivationFunctionType.Exp
    NDIRECT = cfg["direct"]

    with ExitStack() as ctx:
        const = ctx.enter_context(tc.tile_pool(name="const", bufs=1))
        qkp = ctx.enter_context(tc.tile_pool(name="qk", bufs=2))
        epp = ctx.enter_context(tc.tile_pool(name="ep", bufs=2))
        scp = ctx.enter_context(tc.tile_pool(name="sc", bufs=2, space="PSUM"))
        accp = ctx.enter_context(tc.tile_pool(name="accps", bufs=1, space="PSUM"))
        finp = ctx.enter_context(tc.tile_pool(name="fin", bufs=2))

        # ---- load inputs ----
        xp6 = const.tile([6, XP6_FREE], bf16, tag="xp6", name="xp6")
        wts6 = const.tile([6, OC * 2 * 18], bf16, tag="wts6", name="wts6")
        ident = const.tile([128, 128], bf16, tag="ident", name="ident")
        # parallel queues so the first conv's inputs land ASAP
        nc.sync.dma_start(wts6[:], aps["wts6"][:])
        nc.sync.dma_start(ident[:], aps["ident"][:])
        # first piece covers section 0's conv reads (rows 0..13 of the slab)
        P1 = 14 * W132
        nc.scalar.dma_start(xp6[:, 0:P1], aps["xp6"][:, 0:P1])
        nc.gpsimd.dma_start(xp6[:, P1:XP6_FREE], aps["xp6"][:, P1:XP6_FREE])

        def wslot(idx):
            return wts6[:, 128 * idx : 128 * (idx + 1)]

        def conv(pt, dst_off, slot, i, j, row0, nrows):
            """pt[:, dst_off + r*W + w] = conv at out-rows [row0, row0+nrows),
            window offset (i, j); compact 128-wide output."""
            rr = 0
            while rr < nrows:
                sub = min(4, nrows - rr)
                base = (row0 + rr + i) * W132 + j
                rhs = xp6[:, base : base + sub * W132].rearrange(
                    "c (r w) -> c r w", w=W132
                )[:, :, 0:W]
                mm = nc.tensor.matmul(
                    pt[:, dst_off + rr * W : dst_off + (rr + sub) * W],
                    wslot(slot), rhs,
                )
                if rr > 0:
                    mm.ins.ldweights = False
                rr += sub

        def conv_flat(pt, dst_off, slot, base, n):
            """pt[:, dst_off:dst_off+n] = conv over flat xp6 range (132-layout
            output, used for kcv)."""
            off = 0
            while off < n:
                cn = min(512, n - off)
                mm = nc.tensor.matmul(
                    pt[:, dst_off + off : dst_off + off + cn],
                    wslot(slot),
                    xp6[:, base + off : base + off + cn],
                )
                if off > 0:
                    mm.ins.ldweights = False
                off += cn

        # ---- whole-core q / kcv, computed once during the ramp ----
        # q: 32 out-rows compact [128, 4096]; kcv: padded rows 0..34 in
        # 132-layout [128, 4620] plus the 1-element shifted twin (so odd
        # window shifts keep 4B alignment / DVE 2x mode).
        KCVA = (BLK + KS - 1) * W132  # 4620
        QA = BLK * W  # 4096

        def emit_qk_all():
            kcv0 = qkp.tile([128, KCVA + KCV_PAD], bf16, tag="kcv0", name="kcv0")
            kcv1 = qkp.tile([128, KCVA + KCV_PAD], bf16, tag="kcv1", name="kcv1")
            q_t = qkp.tile([128, QA], bf16, tag="q", name="q")
            # kcv first: section 0's logits need its head chunks
            off = 0
            while off < KCVA:
                n = min(HFC, KCVA - off)
                pt = scp.tile([128, HFC], f32, tag="sc", name="ptk")
                conv_flat(pt, 0, 1, off, n)
                nc.scalar.copy(kcv0[:, off : off + n], pt[:, 0:n])
                off += n
            for p0 in range(0, 128, 32):
                nc.sync.dma_start(
                    kcv1[p0 : p0 + 32, 0 : KCVA - 1], kcv0[p0 : p0 + 32, 1:KCVA]
                )
            off = 0
            while off < QA:
                pt = scp.tile([128, HFC], f32, tag="sc", name="ptq")
                conv(pt, 0, 0, PAD, PAD, off // W, NR)
                nc.vector.tensor_copy(q_t[:, off : off + HFC], pt[:])
                off += HFC
            return q_t, kcv0, kcv1

        def logit_pair(qk, row0, ep, pi, ka):
            """ep slots [2pi, 2pi+1] = q * kcv[(i,j)], q * kcv[(i+1,j)] in one
            DVE op: kcv gets a [132,2] outer dim (rows i, i+1), q a
            broadcast dim."""
            qt, kcv0, kcv1 = qk
            i, j = ka // KS, ka % KS
            shift = i * W132 + j
            if shift % 2 == 1:
                ksrc, koff = kcv1, shift - 1
            else:
                ksrc, koff = kcv0, shift
            koff += row0 * W132
            vk = ksrc[:, koff : koff + NR * W132].rearrange(
                "p (r w) -> p r w", w=W132
            )[:, :, 0:W]
            kk = bass.AP(vk.tensor, vk.offset,
                         [list(vk.ap)[0], [W132, 2], *list(vk.ap)[1:]])
            vq = qt[:, row0 * W : row0 * W + HFC].rearrange(
                "p (r w) -> p r w", w=W
            )
            qq = bass.AP(vq.tensor, vq.offset,
                         [list(vq.ap)[0], [0, 2], *list(vq.ap)[1:]])
            out = ep[:, 2 * pi * HFC : (2 * pi + 2) * HFC].rearrange(
                "p (s r w) -> p s r w", s=2, w=W
            )
            nc.vector.tensor_mul(out, qq, kk)

        # ---- main loop ----

        def emit_finals(s, s_ps, num_ps):
            rinv = finp.tile([128, HFC], f32, tag="rinv", name="rinv")
            nc.vector.reciprocal_approx_fast(rinv[:], s_ps[:])
            o_t = finp.tile([128, HFC], f32, tag="o", name="o")
            nc.vector.tensor_mul(o_t[:], num_ps[:], rinv[:])
            o_v = o_t[:].rearrange("p (r w) -> p r w", w=W)
            for b in (0, 1):
                nc.sync.dma_start(
                    aps["out"][:, b * BLK + s * NR : b * BLK + (s + 1) * NR, :],
                    o_v[64 * b : 64 * (b + 1)],
                )

        def emit_accs(g, ep_g, p_g, s_ps, num_ps):
            """psum_s += I @ e, psum_num += I @ p for group g's tiles.
            The identity stays resident in the PE array across the burst."""
            first = True
            for src, dst in ((ep_g, s_ps), (p_g, num_ps)):
                for c in range(8):
                    mm = nc.tensor.matmul(
                        dst[:, (c % 2) * 512 : (c % 2 + 1) * 512],
                        ident[:],
                        src[:, c * 512 : (c + 1) * 512],
                        start=(g == 0 and c < 2),
                        stop=(g == 3 and c >= 6),
                        skip_group_check=True,
                    )
                    if not first:
                        mm.ins.ldweights = False
                    first = False

        # pending_accs: (g, ep_g, p_g, s_ps, num_ps) emitted one group late so
        # the acc burst fills the PE window where this group's later value
        # convs wait on PSUM-scratch reuse (keeps the PE p-state ramped).
        pending_accs = None
        pending_finals = None
        qk = emit_qk_all()
        for s in range(NSEC):
            row0 = s * NR

            ep = epp.tile([128, 4 * HFC], bf16, tag="ep", name="ep")
            for pi, (ka, _kb) in enumerate(GROUPS[0]):
                logit_pair(qk, row0, ep, pi, ka)

            s_ps = num_ps = None
            for g in range(4):
                group = GROUPS[g]
                slots = [k for pair in group for k in pair]  # [ka,ka+4,kb,kb+4]
                # evac groups first: ScalarE has slack early in the section;
                # direct (PSUM-read) groups close the section so the last
                # val-muls never wait on ScalarE copies.
                direct = g >= 4 - NDIRECT
                nc.scalar.activation(ep[:], ep[:], EXP)

                # value convs -> PSUM, first half (scratch slots 0-1)
                vs_ps = []
                for idx in (0, 1):
                    k = slots[idx]
                    pt = scp.tile([128, HFC], f32, tag="sc", name="vs")
                    conv(pt, 0, 2 + k, k // KS, k % KS, row0, NR)
                    vs_ps.append(pt)

                # last group's acc burst: ready PE work that covers the
                # scratch-WAR stall before slots 2-3 can start
                if pending_accs is not None:
                    emit_accs(*pending_accs)
                    pending_accs = None
                if g == 0:
                    # prev section's normalize+store, then (re)claim the acc
                    # banks (alloc after those reads so WAR deps are tracked)
                    if pending_finals is not None:
                        emit_finals(*pending_finals)
                        pending_finals = None
                    s_ps = accp.tile([128, HFC], f32, tag="s", name="s_ps")
                    num_ps = accp.tile([128, HFC], f32, tag="num", name="num_ps")

                # value convs, second half
                for idx in (2, 3):
                    k = slots[idx]
                    pt = scp.tile([128, HFC], f32, tag="sc", name="vs")
                    conv(pt, 0, 2 + k, k // KS, k % KS, row0, NR)
                    vs_ps.append(pt)

                # next group's logits (DVE) ahead of this group's val-mul
                ep_prev = ep
                if g < 3:
                    ep = epp.tile([128, 4 * HFC], bf16, tag="ep", name="ep")
                    for pi, (ka, _kb) in enumerate(GROUPS[g + 1]):
                        logit_pair(qk, row0, ep, pi, ka)

                # val-mul: p = e * vs, per k-slot so each mul waits on only
                # one evacuation / conv
                p_t = epp.tile([128, 4 * HFC], bf16, tag="p", name="p")
                vse = None
                if not direct:
                    vse = epp.tile([128, 4 * HFC], bf16, tag="vse", name="vse")
                for idx in range(4):
                    if direct:
                        src = vs_ps[idx][:]
                    else:
                        nc.scalar.copy(
                            vse[:, idx * HFC : (idx + 1) * HFC], vs_ps[idx][:]
                        )
                        src = vse[:, idx * HFC : (idx + 1) * HFC]
                    nc.vector.tensor_mul(
                        p_t[:, idx * HFC : (idx + 1) * HFC],
                        ep_prev[:, idx * HFC : (idx + 1) * HFC],
                        src,
                    )

                pending_accs = (g, ep_prev, p_t, s_ps, num_ps)

            pending_finals = (s, s_ps, num_ps)

        emit_accs(*pending_accs)
        emit_finals(*pending_finals)


def _build(cfg):
    key = tuple(sorted(cfg.items()))
    if key in _CACHE:
        return _CACHE[key]
    import concourse.tile as tile
    from concourse import bacc, mybir

    nc = bacc.Bacc(
        "TRN2", target_bir_lowering=False, debug=False, num_devices=NCORES
    )
    f32 = mybir.dt.float32
    bf16 = mybir.dt.bfloat16
    aps = {}
    aps["xp6"] = nc.dram_tensor("xp6", [6, XP6_FREE], bf16,
                                kind="ExternalInput").ap()
    aps["wts6"] = nc.dram_tensor("wts6", [6, OC * 2 * 18], bf16,
                                 kind="ExternalInput").ap()
    aps["ident"] = nc.dram_tensor("ident", [128, 128], bf16,
                                  kind="ExternalInput").ap()
    aps["out"] = nc.dram_tensor("out", [OC, SH_OUT_ROWS, W], f32,
                                kind="ExternalOutput").ap()

    with tile.TileContext(nc) as tc:
        _emit(nc, tc, aps, cfg)
    nc.compile()
    _CACHE[key] = nc
    return nc


def _host_prep(inputs, cfg):
    import ml_dtypes

    x = np.asarray(inputs["x"], np.float32)
    key_w = np.asarray(inputs["key_w"], np.float32)
    query_w = np.asarray(inputs["query_w"], np.float32)
    value_w = np.asarray(inputs["value_w"], np.float32)
    emb_a = np.asarray(inputs["emb_a"], np.float32)
    emb_b = np.asarray(inputs["emb_b"], np.float32)
    emb_mix = np.asarray(inputs["emb_mix"], np.float32)

    # emb softmax over m, then effective per-offset value matrices W_k [16,64,3]
    la = emb_mix @ emb_a  # (M, KS)
    lb = emb_mix @ emb_b  # (M, KS)
    eloG = (la[:, :, None] + lb[:, None, :]).reshape(M, KS * KS).astype(np.float64)
    eloG -= eloG.max(axis=0, keepdims=True)
    emb = np.exp(eloG)
    emb /= emb.sum(axis=0, keepdims=True)  # (M, 16)
    wk = np.einsum("mk,moc->koc", emb.astype(np.float32), value_w)  # (16,64,3)

    # wts6 [6, 128*18]: block-diagonal (block0 chans on rows 0-2 -> parts 0-63,
    # block1 chans on rows 3-5 -> parts 64-127)
    wts6 = np.zeros((6, 128 * 18), np.float32)
    mats = [query_w, key_w] + [wk[k] for k in range(NK)]
    for idx, mat in enumerate(mats):
        wts6[0:3, 128 * idx : 128 * idx + 64] = mat.T
        wts6[3:6, 128 * idx + 64 : 128 * (idx + 1)] = mat.T

    ident = np.eye(128, dtype=np.float32)

    # padded input, shards
    xp = np.zeros((B, IC, H + 2 * PAD, W + 2 * PAD), np.float32)
    xp[:, :, PAD : PAD + H, PAD : PAD + W] = x

    in_maps = []
    for c in range(NCORES):
        b, hh = c // 2, c % 2
        sh = xp[b, :, hh * SH_OUT_ROWS : hh * SH_OUT_ROWS + SH_ROWS, :]
        slab6 = np.zeros((6, XP6_FREE), np.float32)
        slab6[0:3, : SH_ROWS * W132] = sh.reshape(IC, -1)
        # rows 32.. (block1's conv window) shifted onto partitions 3-5
        slab6[3:6, : (SH_ROWS - BLK) * W132] = sh[:, BLK:, :].reshape(IC, -1)
        in_maps.append({
            "xp6": slab6.astype(ml_dtypes.bfloat16),
            "wts6": wts6.astype(ml_dtypes.bfloat16),
            "ident": ident.astype(ml_dtypes.bfloat16),
        })
    return in_maps


def _ensure_ntff_hook():
    """The agent image's antenv lacks axon_hooks, so boot() could not register
    the NTFF profile hook. Inject the registry module and register the
    ctypes-based hook so run_bass_kernel_spmd(trace=True) can profile."""
    import types

    try:
        import antenv
    except ImportError:
        return
    if "antenv.axon_hooks" in sys.modules:
        return
    try:
        from trn_agent_boot.trn_boot import _ntff_profile_via_ctypes

        hook = _ntff_profile_via_ctypes("/opt/axon/libaxon_pjrt.so")
    except Exception:
        hook = None
    mod = types.ModuleType("antenv.axon_hooks")
    mod._hook = hook
    mod.set_axon_ntff_profile_hook = lambda h: setattr(mod, "_hook", h)
    mod.get_axon_ntff_profile_hook = lambda: mod._hook
    sys.modules["antenv.axon_hooks"] = mod
    antenv.axon_hooks = mod


def kernel(**inputs):
    global LAST_RESULT
    cfg = dict(CFG)
    in_maps = _host_prep(inputs, cfg)
    nc = _build(cfg)

    from concourse.bass_utils import run_bass_kernel_spmd

    trace = os.environ.get("KERNEL_TRACE", "0") == "1"
    if trace:
        _ensure_ntff_hook()
    res = run_bass_kernel_spmd(
        nc, in_maps, core_ids=list(range(NCORES)), trace=trace
    )
    LAST_RESULT = res

    out = np.empty((B, OC, H, W), np.float32)
    for c in range(NCORES):
        b, hh = c // 2, c % 2
        out[b, :, hh * SH_OUT_ROWS : (hh + 1) * SH_OUT_ROWS, :] = res.results[c]["out"]
    return out
